# revision 34
# baseline (speedup 1.0000x reference)
"""Trainium2 Bass kernel for Performer-style causal attention (FAVOR+).

Reference per (b,h) slice, S=1024, D=M=64:
    qp = exp(DN*q@P - 0.5*DN^2*||q||^2 - rowmax(DN*q@P)) + eps          [S,M]
    kp = exp(DN*k@P - 0.5*DN^2*||k||^2 - globalmax(DN*k@P)) + eps       [S,M]
    s  = tril(qp @ kp^T);  out = (s / rowsum(s)) @ v                    [S,D]

Strategy: 64 (b,h) pairs sharded over 8 cores (8 heads/core).  Host
pre-transposes q,k to [d,s] bf16 (layout prep only), pads v with a ones
column, and permutes the output back.  On-device per head:
  - dash = q@P via bf16 matmuls from the preloaded qT/kT (no PE transposes
    of raw data);
  - the exp bias (-diag - stabilizer), computed from a PE row-norm matmul
    and a DVE rowmax, is added to dash inside the PE via a rank-1 f32r
    matmul accumulate, so exp is a single batched activation per tensor;
  - chunked causal attention (8 chunks of 128 rows) with per-chunk delta
    states kp_c^T@[v|1] computed independently (no serial prefix chain);
    chunk outputs get the diagonal-block tril product plus qp_c @ (block
    state + deltas), normalized by the ones-column.
"""

import numpy as np
import ml_dtypes

import concourse.bass as bass
import concourse.bass_isa as bass_isa
import concourse.bacc as bacc
import concourse.mybir as mybir
import concourse.tile as tile
from concourse.bass_utils import run_bass_kernel_spmd
from concourse.masks import make_identity

F32 = mybir.dt.float32
F32R = mybir.dt.float32r
BF16 = mybir.dt.bfloat16
EPS = 1e-4

B, H, S, D, M = 4, 16, 1024, 64, 64
NCORES = 8
HPC = B * H // NCORES          # heads per core
C = 128                        # chunk rows
T = S // C                     # chunks per head
DN = D ** -0.25                # data normalizer
AF = mybir.ActivationFunctionType
AL = mybir.AluOpType


def build_kernel():
    nc = bacc.Bacc()
    qk_d = nc.declare_dram_parameter("qkT", [HPC, 2 * D, S], BF16, isOutput=False)
    v_d = nc.declare_dram_parameter("v", [HPC, C, T, D + 1], BF16, isOutput=False)
    p_d = nc.declare_dram_parameter("proj", [2 * D, M], BF16, isOutput=False)
    o_d = nc.declare_dram_parameter("out", [HPC, C, T, D], F32, isOutput=True)

    with tile.TileContext(nc) as tc:
        with (
            tc.tile_pool(name="const", bufs=1) as const,
            tc.tile_pool(name="io", bufs=5) as io,
            tc.tile_pool(name="feat", bufs=2) as feat,
            tc.tile_pool(name="small", bufs=3) as small,
            tc.tile_pool(name="psD", bufs=1, space="PSUM") as psD,
            tc.tile_pool(name="psT", bufs=1, space="PSUM") as psT,
            tc.tile_pool(name="psA", bufs=1, space="PSUM") as psA,
            tc.tile_pool(name="psO", bufs=2, space="PSUM") as psO,
        ):
            # ---- constants -------------------------------------------------
            identb = const.tile([128, 128], BF16)
            identf = const.tile([128, 128], F32)
            make_identity(nc, identf)
            nc.vector.tensor_copy(identb, identf)
            # projection, stacked twice on partitions (q rows 0:64, k 64:128)
            proj2 = const.tile([2 * D, M], BF16)
            nc.sync.dma_start(out=proj2, in_=p_d[:, :])
            # negcol: col0 = -1 on partitions 0:64, col1 = -1 on 64:128
            negcol = const.tile([128, 2], BF16)
            nc.gpsimd.memset(negcol, -1.0)
            nc.gpsimd.affine_select(
                out=negcol, in_=negcol, compare_op=AL.is_ge,
                fill=0.0, base=0, pattern=[[-D, 2]], channel_multiplier=1)
            nc.gpsimd.affine_select(
                out=negcol, in_=negcol, compare_op=AL.is_ge,
                fill=0.0, base=D - 1, pattern=[[D, 2]],
                channel_multiplier=-1)
            # causal mask for a group of 4 diagonal blocks ([128, 4, 128]):
            # mask[p, g, j] = 1 iff j >= p  (keep key t <= query i)
            mask4 = const.tile([128, 4, 128], F32)
            nc.gpsimd.memset(mask4, 1.0)
            nc.gpsimd.affine_select(
                out=mask4, in_=mask4, compare_op=AL.is_ge,
                fill=0.0, base=0, pattern=[[0, 4], [1, 128]],
                channel_multiplier=-1)

            st = {}

            def emit_L(h):
                qkT = io.tile([2 * D, S], BF16, tag="qkT")
                vaug = io.tile([C, T, D + 1], BF16, tag="vaug")
                nc.sync.dma_start(out=qkT, in_=qk_d[h])
                nc.sync.dma_start(out=vaug, in_=v_d[h])
                st[h] = {"qkT": qkT, "vaug": vaug}

            def emit_SQ(h):
                """squares for the row norms (off the critical path)"""
                d = st[h]
                sq = feat.tile([2 * D, S], BF16, tag="sq")
                nc.gpsimd.tensor_tensor(sq, d["qkT"], d["qkT"], op=AL.mult)
                d["sq"] = sq

            def emit_F1(h):
                """dash matmuls + diag + stabilizer chain + raw exps"""
                d = st[h]
                qkT = d["qkT"]
                sq = d["sq"]
                # scratch PSUM bank shared with delta/sblk: dg at [392:408)
                scr = psA.tile([128, 512], F32, tag="scrA")
                d["scr"] = scr
                dg = scr[:, 392:408].rearrange("p (c t) -> p c t", t=2)
                dq = psD.tile([C, T, M], F32, tag="dq", bufs=1)
                dk = psD.tile([C, T, M], F32, tag="dk", bufs=1)
                for c in range(T):
                    sl = slice(c * C, (c + 1) * C)
                    nc.tensor.matmul(dk[:, c, :], qkT[D:2 * D, sl],
                                     proj2[D:2 * D, :],
                                     start=True, stop=True,
                                     skip_group_check=True)
                    nc.tensor.matmul(dq[:, c, :], qkT[0:D, sl], proj2[0:D, :],
                                     start=True, stop=True,
                                     skip_group_check=True)
                for c in range(T):
                    sl = slice(c * C, (c + 1) * C)
                    # diag[:, c, 0] = -||q_i||^2 ; [:, c, 1] = -||k_i||^2
                    nc.tensor.matmul(dg[:, c, :], sq[:, sl], negcol,
                                     start=True, stop=True,
                                     skip_group_check=True)
                # raw row max for k only (q needs no stabilizer: a per-row
                # scale cancels in the normalization; measured rel err 7e-4)
                rmk = small.tile([C, T], F32, tag="rmk")
                nc.vector.reduce_max(rmk, dk, axis=mybir.AxisListType.X)
                # k global max (scalar per head)
                km1 = small.tile([C, 1], F32, tag="km1")
                nc.vector.reduce_max(km1, rmk, axis=mybir.AxisListType.X)
                kg = small.tile([C, 1], F32, tag="kg")
                nc.gpsimd.partition_all_reduce(kg, km1, 128,
                                               bass_isa.ReduceOp.max)
                kgs = small.tile([C, 1], F32, tag="kgs")
                nc.vector.tensor_scalar(kgs, kg, -DN, None, op0=AL.mult)
                # raw exps (batched) first so the folds unblock early, then the
                # row scales gq = exp(-diag_q), hk = exp(-diag_k - stab)
                xq = feat.tile([C, T, M], BF16, tag="xq")
                xk = feat.tile([C, T, M], BF16, tag="xk")
                nc.scalar.activation(xk, dk, AF.Exp, scale=DN)
                nc.scalar.activation(xq, dq, AF.Exp, scale=DN)
                gq = small.tile([C, T], F32, tag="gq")
                hk = small.tile([C, T], F32, tag="hk")
                nc.scalar.activation(hk, dg[:, :, 1], AF.Exp,
                                     scale=0.5 * DN * DN, bias=kgs[:, 0:1])
                nc.scalar.activation(gq, dg[:, :, 0], AF.Exp, scale=0.5 * DN * DN)
                d["xq"], d["xk"], d["gq"], d["hk"] = xq, xk, gq, hk

            def emit_F1b(h):
                d = st[h]
                xq, xk, gq, hk = d["xq"], d["xk"], d["gq"], d["hk"]
                qpe = feat.tile([C, T, M], BF16, tag="qpe")
                kpe = feat.tile([C, T, M], BF16, tag="kpe")
                for c in range(T):
                    nc.vector.tensor_scalar(kpe[:, c, :], xk[:, c, :],
                                            hk[:, c:c + 1], EPS,
                                            op0=AL.mult, op1=AL.add)
                    nc.gpsimd.tensor_scalar(qpe[:, c, :], xq[:, c, :],
                                            gq[:, c:c + 1], EPS,
                                            op0=AL.mult, op1=AL.add)
                d["qpe"], d["kpe"] = qpe, kpe

            def emit_F2(h):
                """feature transposes (single chunks, everything at partition
                base 0): qpT_sb/kpT_sb [64, 8, 128], cols = row index i."""
                d = st[h]
                tpq = psT.tile([M, T, 128], BF16, tag="tpq")
                tpk = psT.tile([M, T, 128], BF16, tag="tpk")
                for c in range(T):
                    nc.tensor.transpose(tpq[:, c, :], d["qpe"][:, c, :],
                                        identb)
                    nc.tensor.transpose(tpk[:, c, :], d["kpe"][:, c, :],
                                        identb)
                qpT_sb = feat.tile([M, T, 128], BF16, tag="qpT_sb")
                kpT_sb = feat.tile([M, T, 128], BF16, tag="kpT_sb")
                nc.scalar.copy(qpT_sb, tpq)
                nc.scalar.copy(kpT_sb, tpk)
                d["qpT"], d["kpT"] = qpT_sb, kpT_sb

            def qpT(h, c):
                return st[h]["qpT"][:, c, :]

            def kpT(h, c):
                return st[h]["kpT"][:, c, :]

            def emit_B(h):
                d = st[h]
                vaug = d["vaug"]
                kpe = d["kpe"]
                # --- per-chunk delta states (independent, all at base 0).
                # delta 3 is only ever part of the block state, so skipped:
                # slots = chunks [0, 1, 2, 4, 5, 6]
                dl_ps = d["scr"][0:M, 0:390].rearrange("p (c d) -> p c d", d=D + 1)
                for si, c in enumerate((0, 1, 2, 4, 5, 6)):
                    nc.tensor.matmul(dl_ps[:, si, :], kpe[:, c, :],
                                     vaug[:, c, :], start=True, stop=True,
                                     skip_group_check=True)
                delta = small.tile([M, 6, D + 1], BF16, tag="deltas")
                nc.scalar.copy(delta, dl_ps)
                # block state = sum of chunk 0..3 deltas (psum accumulate)
                sb_ps = d["scr"][0:M, 408:408 + D + 1]
                for c in range(4):
                    nc.tensor.matmul(sb_ps, kpe[:, c, :], vaug[:, c, :],
                                     start=(c == 0), stop=(c == 3),
                                     skip_group_check=True)
                sblk = small.tile([M, D + 1], BF16, tag="sblks")
                nc.vector.tensor_copy(sblk, sb_ps)

                o_out = io.tile([C, T, D], F32, tag="oout")
                for g in range(2):
                    # diagonal blocks
                    sT = psT.tile([C, 4, C], F32, tag="sT")
                    for j in range(4):
                        c = 4 * g + j
                        nc.tensor.matmul(sT[:, j, :], kpT(h, c), qpT(h, c),
                                         start=True, stop=True,
                                         skip_group_check=True)
                    pT = feat.tile([C, 4, C], BF16, tag="pT")
                    nc.vector.tensor_tensor(pT, sT, mask4, op=AL.mult)
                    o_ps = psO.tile([C, 4, D + 1], F32, tag="o")
                    for j in range(4):
                        c = 4 * g + j
                        nc.tensor.matmul(o_ps[:, j, :], pT[:, j, :],
                                         vaug[:, c, :],
                                         start=True, stop=(c == 0),
                                         skip_group_check=True)
                        if g == 1:
                            ndel = c - 4
                            nc.tensor.matmul(o_ps[:, j, :], qpT(h, c), sblk,
                                             start=False, stop=(ndel == 0),
                                             skip_group_check=True)
                            for ci in range(4, c):
                                nc.tensor.matmul(
                                    o_ps[:, j, :], qpT(h, c),
                                    delta[:, ci - 1, :],
                                    start=False, stop=(ci == c - 1),
                                    skip_group_check=True)
                        else:
                            for ci in range(c):
                                nc.tensor.matmul(
                                    o_ps[:, j, :], qpT(h, c),
                                    delta[:, ci, :],
                                    start=False, stop=(ci == c - 1),
                                    skip_group_check=True)
                    # normalize and write into o_out
                    rcp = small.tile([C, 4], F32, tag="rcp")
                    nc.vector.reciprocal(rcp, o_ps[:, :, D:D + 1])
                    nc.vector.tensor_tensor(o_out[:, 4 * g:4 * g + 4, :],
                                       o_ps[:, :, 0:D],
                                       rcp.to_broadcast((C, 4, D)),
                                       op=AL.mult)
                nc.scalar.dma_start(out=o_d[h], in_=o_out)

            for i in range(HPC + 2):
                if i < HPC:
                    emit_L(i)
                if 1 <= i <= HPC:
                    emit_F1(i - 1)
                if 2 <= i:
                    emit_B(i - 2)
                if 1 <= i <= HPC:
                    emit_F1b(i - 1)
                    emit_F2(i - 1)
                if i < HPC:
                    emit_SQ(i)
    nc.finalize()
    return nc


def make_in_maps(q, k, v, projection_matrix):
    qf = np.asarray(q, dtype=np.float32).reshape(B * H, S, D)
    kf = np.asarray(k, dtype=np.float32).reshape(B * H, S, D)
    vf = np.asarray(v, dtype=np.float32).reshape(B * H, S, D)
    pf = np.asarray(projection_matrix, dtype=np.float32)

    qkT = np.empty((B * H, 2 * D, S), dtype=ml_dtypes.bfloat16)
    qkT[:, 0:D, :] = qf.transpose(0, 2, 1).astype(ml_dtypes.bfloat16)
    qkT[:, D:2 * D, :] = kf.transpose(0, 2, 1).astype(ml_dtypes.bfloat16)
    # v: [h, s, d] -> [h, p, c, d+1] with ones column baked in
    vp = np.empty((B * H, C, T, D + 1), dtype=ml_dtypes.bfloat16)
    vp[:, :, :, 0:D] = vf.reshape(B * H, T, C, D).transpose(0, 2, 1, 3) \
        .astype(ml_dtypes.bfloat16)
    vp[:, :, :, D] = np.ones((), dtype=ml_dtypes.bfloat16)
    p2 = np.concatenate([pf, pf], axis=0).astype(ml_dtypes.bfloat16)

    in_maps = []
    for core in range(NCORES):
        sl = slice(core * HPC, (core + 1) * HPC)
        in_maps.append({"qkT": np.ascontiguousarray(qkT[sl]),
                        "v": np.ascontiguousarray(vp[sl]),
                        "proj": p2})
    return in_maps


_NC_CACHE = None


def kernel(q, k, v, projection_matrix):
    global _NC_CACHE
    if _NC_CACHE is None:
        _NC_CACHE = build_kernel()
    nc = _NC_CACHE

    in_maps = make_in_maps(q, k, v, projection_matrix)
    res = run_bass_kernel_spmd(nc, in_maps, list(range(NCORES)))
    out = np.concatenate([r["out"] for r in res.results], axis=0)
    # [h, p, c, d] -> [h, c*128+p, d]
    out = out.transpose(0, 2, 1, 3).reshape(B, H, S, D)
    return np.ascontiguousarray(out)


if __name__ == "__main__":
    rng = np.random.default_rng(0)
    inputs = {
        "q": rng.standard_normal((B, H, S, D)).astype(np.float32),
        "k": rng.standard_normal((B, H, S, D)).astype(np.float32),
        "v": rng.standard_normal((B, H, S, D)).astype(np.float32),
        "projection_matrix":
            (rng.standard_normal((D, M)) / np.sqrt(M)).astype(np.float32),
    }
    out = kernel(**inputs)
    print(out.shape, out.dtype)


# revision 35
# speedup vs baseline: 1.2590x; 1.2590x over previous
"""Trainium2 Bass kernel for Performer-style causal attention (FAVOR+).

Reference per (b,h) slice, S=1024, D=M=64:
    qp = exp(DN*q@P - 0.5*DN^2*||q||^2 - rowmax(DN*q@P)) + eps          [S,M]
    kp = exp(DN*k@P - 0.5*DN^2*||k||^2 - globalmax(DN*k@P)) + eps       [S,M]
    s  = tril(qp @ kp^T);  out = (s / rowsum(s)) @ v                    [S,D]

Strategy: 64 (b,h) pairs sharded over 8 cores (8 heads/core).  Host
pre-transposes q,k to [d,s] bf16 (layout prep only), pads v with a ones
column, and permutes the output back.  On-device per head:
  - dash = q@P via bf16 matmuls from the preloaded qT/kT (no PE transposes
    of raw data);
  - the exp bias (-diag - stabilizer), computed from a PE row-norm matmul
    and a DVE rowmax, is added to dash inside the PE via a rank-1 f32r
    matmul accumulate, so exp is a single batched activation per tensor;
  - chunked causal attention (8 chunks of 128 rows) with per-chunk delta
    states kp_c^T@[v|1] computed independently (no serial prefix chain);
    chunk outputs get the diagonal-block tril product plus qp_c @ (block
    state + deltas), normalized by the ones-column.
"""

import numpy as np
import ml_dtypes

import concourse.bass as bass
import concourse.bass_isa as bass_isa
import concourse.bacc as bacc
import concourse.mybir as mybir
import concourse.tile as tile
from concourse.bass_utils import run_bass_kernel_spmd
from concourse.masks import make_identity

F32 = mybir.dt.float32
F32R = mybir.dt.float32r
BF16 = mybir.dt.bfloat16
EPS = 1e-4

B, H, S, D, M = 4, 16, 1024, 64, 64
NCORES = 8
HPC = B * H // NCORES          # heads per core
C = 128                        # chunk rows
T = S // C                     # chunks per head
DN = D ** -0.25                # data normalizer
AF = mybir.ActivationFunctionType
AL = mybir.AluOpType


def build_kernel():
    nc = bacc.Bacc()
    qk_d = nc.declare_dram_parameter("qkT", [HPC, 2 * D, S], BF16, isOutput=False)
    v_d = nc.declare_dram_parameter("v", [HPC, C, T, D + 1], BF16, isOutput=False)
    p_d = nc.declare_dram_parameter("proj", [2 * D, M], BF16, isOutput=False)
    o_d = nc.declare_dram_parameter("out", [HPC, C, T, D], F32, isOutput=True)

    with tile.TileContext(nc) as tc:
        with (
            tc.tile_pool(name="const", bufs=1) as const,
            tc.tile_pool(name="io", bufs=5) as io,
            tc.tile_pool(name="feat", bufs=2) as feat,
            tc.tile_pool(name="small", bufs=3) as small,
            tc.tile_pool(name="psD", bufs=1, space="PSUM") as psD,
            tc.tile_pool(name="psT", bufs=1, space="PSUM") as psT,
            tc.tile_pool(name="psA", bufs=1, space="PSUM") as psA,
            tc.tile_pool(name="psO", bufs=2, space="PSUM") as psO,
        ):
            # ---- constants -------------------------------------------------
            identb = const.tile([128, 128], BF16)
            identf = const.tile([128, 128], F32)
            make_identity(nc, identf)
            nc.vector.tensor_copy(identb, identf)
            # projection, stacked twice on partitions (q rows 0:64, k 64:128)
            proj2 = const.tile([2 * D, M], BF16)
            nc.sync.dma_start(out=proj2, in_=p_d[:, :])
            # negcol: -1 on partitions 64:128 (k half), 0 elsewhere
            negcol = const.tile([128, 1], BF16)
            nc.gpsimd.memset(negcol, -1.0)
            nc.gpsimd.affine_select(
                out=negcol, in_=negcol, compare_op=AL.is_ge,
                fill=0.0, base=-D, pattern=[[0, 1]], channel_multiplier=1)
            # causal mask for a group of 4 diagonal blocks ([128, 4, 128]):
            # mask[p, g, j] = 1 iff j >= p  (keep key t <= query i)
            mask4 = const.tile([128, 4, 128], F32)
            nc.gpsimd.memset(mask4, 1.0)
            nc.gpsimd.affine_select(
                out=mask4, in_=mask4, compare_op=AL.is_ge,
                fill=0.0, base=0, pattern=[[0, 4], [1, 128]],
                channel_multiplier=-1)

            st = {}

            def emit_L(h):
                qkT = io.tile([2 * D, S], BF16, tag="qkT")
                vaug = io.tile([C, T, D + 1], BF16, tag="vaug")
                nc.sync.dma_start(out=qkT, in_=qk_d[h])
                nc.sync.dma_start(out=vaug, in_=v_d[h])
                st[h] = {"qkT": qkT, "vaug": vaug}

            def emit_SQ(h):
                """squares for the row norms (off the critical path)"""
                d = st[h]
                sq = feat.tile([2 * D, S], BF16, tag="sq")
                nc.gpsimd.tensor_tensor(sq, d["qkT"], d["qkT"], op=AL.mult)
                d["sq"] = sq

            def emit_F1(h):
                """k: dash + stabilizer chain + raw exp.  q: transposed-layout
                dash (P^T @ qT) + raw exp = qpT directly."""
                d = st[h]
                qkT = d["qkT"]
                sq = d["sq"]
                # scratch PSUM bank shared with delta/sblk: dg at [392:400)
                scr = psA.tile([128, 512], F32, tag="scrA")
                d["scr"] = scr
                dg = scr[:, 392:400]
                dk = psD.tile([C, T, M], F32, tag="dk", bufs=1)
                for c in range(T):
                    sl = slice(c * C, (c + 1) * C)
                    nc.tensor.matmul(dk[:, c, :], qkT[D:2 * D, sl],
                                     proj2[D:2 * D, :],
                                     start=True, stop=True,
                                     skip_group_check=True)
                dtq1 = psD.tile([M, S // 2], F32, tag="dtq1", bufs=1)
                dtq2 = psD.tile([M, S // 2], F32, tag="dtq2", bufs=1)
                nc.tensor.matmul(dtq1, proj2[0:D, :], qkT[0:D, 0:S // 2],
                                 start=True, stop=True, skip_group_check=True)
                nc.tensor.matmul(dtq2, proj2[0:D, :], qkT[0:D, S // 2:S],
                                 start=True, stop=True, skip_group_check=True)
                for c in range(T):
                    sl = slice(c * C, (c + 1) * C)
                    # dg[:, c] = -||k_i||^2
                    nc.tensor.matmul(dg[:, c:c + 1], sq[:, sl], negcol,
                                     start=True, stop=True,
                                     skip_group_check=True)
                # raw row max for k (global stabilizer)
                rmk = small.tile([C, T], F32, tag="rmk")
                nc.vector.reduce_max(rmk, dk, axis=mybir.AxisListType.X)
                km1 = small.tile([C, 1], F32, tag="km1")
                nc.vector.reduce_max(km1, rmk, axis=mybir.AxisListType.X)
                kg = small.tile([C, 1], F32, tag="kg")
                nc.gpsimd.partition_all_reduce(kg, km1, 128,
                                               bass_isa.ReduceOp.max)
                kgs = small.tile([C, 1], F32, tag="kgs")
                nc.vector.tensor_scalar(kgs, kg, -DN, None, op0=AL.mult)
                # raw exps; xqT IS the final transposed q feature map
                xk = feat.tile([C, T, M], BF16, tag="xk")
                nc.scalar.activation(xk, dk, AF.Exp, scale=DN)
                xqT = feat.tile([M, S], BF16, tag="xqT")
                nc.scalar.activation(xqT[:, 0:S // 2], dtq1, AF.Exp, scale=DN)
                nc.scalar.activation(xqT[:, S // 2:S], dtq2, AF.Exp, scale=DN)
                hk = small.tile([C, T], F32, tag="hk")
                nc.scalar.activation(hk, dg, AF.Exp,
                                     scale=0.5 * DN * DN, bias=kgs[:, 0:1])
                d["xk"], d["hk"], d["xqT"] = xk, hk, xqT

            def emit_F1b(h):
                d = st[h]
                xk, hk = d["xk"], d["hk"]
                kpe = feat.tile([C, T, M], BF16, tag="kpe")
                for c in range(T):
                    nc.vector.tensor_scalar(kpe[:, c, :], xk[:, c, :],
                                            hk[:, c:c + 1], EPS,
                                            op0=AL.mult, op1=AL.add)
                d["kpe"] = kpe

            def emit_F2(h):
                """k feature transposes (single chunks, base 0)."""
                d = st[h]
                tpk = psT.tile([M, T, 128], BF16, tag="tpk")
                for c in range(T):
                    nc.tensor.transpose(tpk[:, c, :], d["kpe"][:, c, :],
                                        identb)
                kpT_sb = feat.tile([M, T, 128], BF16, tag="kpT_sb")
                nc.scalar.copy(kpT_sb, tpk)
                d["kpT"] = kpT_sb

            def qpT(h, c):
                return st[h]["xqT"][:, c * C:(c + 1) * C]

            def kpT(h, c):
                return st[h]["kpT"][:, c, :]

            def emit_B(h):
                d = st[h]
                vaug = d["vaug"]
                kpe = d["kpe"]
                # --- per-chunk delta states (independent, all at base 0).
                # delta 3 is only ever part of the block state, so skipped:
                # slots = chunks [0, 1, 2, 4, 5, 6]
                dl_ps = d["scr"][0:M, 0:390].rearrange("p (c d) -> p c d", d=D + 1)
                for si, c in enumerate((0, 1, 2, 4, 5, 6)):
                    nc.tensor.matmul(dl_ps[:, si, :], kpe[:, c, :],
                                     vaug[:, c, :], start=True, stop=True,
                                     skip_group_check=True)
                delta = small.tile([M, 6, D + 1], BF16, tag="deltas")
                nc.scalar.copy(delta, dl_ps)
                # block state = sum of chunk 0..3 deltas (psum accumulate)
                sb_ps = d["scr"][0:M, 408:408 + D + 1]
                for c in range(4):
                    nc.tensor.matmul(sb_ps, kpe[:, c, :], vaug[:, c, :],
                                     start=(c == 0), stop=(c == 3),
                                     skip_group_check=True)
                sblk = small.tile([M, D + 1], BF16, tag="sblks")
                nc.vector.tensor_copy(sblk, sb_ps)

                o_out = io.tile([C, T, D], F32, tag="oout")
                for g in range(2):
                    # diagonal blocks
                    sT = psT.tile([C, 4, C], F32, tag="sT")
                    for j in range(4):
                        c = 4 * g + j
                        nc.tensor.matmul(sT[:, j, :], kpT(h, c), qpT(h, c),
                                         start=True, stop=True,
                                         skip_group_check=True)
                    pT = feat.tile([C, 4, C], BF16, tag="pT")
                    nc.vector.tensor_tensor(pT, sT, mask4, op=AL.mult)
                    o_ps = psO.tile([C, 4, D + 1], F32, tag="o")
                    for j in range(4):
                        c = 4 * g + j
                        nc.tensor.matmul(o_ps[:, j, :], pT[:, j, :],
                                         vaug[:, c, :],
                                         start=True, stop=(c == 0),
                                         skip_group_check=True)
                        if g == 1:
                            ndel = c - 4
                            nc.tensor.matmul(o_ps[:, j, :], qpT(h, c), sblk,
                                             start=False, stop=(ndel == 0),
                                             skip_group_check=True)
                            for ci in range(4, c):
                                nc.tensor.matmul(
                                    o_ps[:, j, :], qpT(h, c),
                                    delta[:, ci - 1, :],
                                    start=False, stop=(ci == c - 1),
                                    skip_group_check=True)
                        else:
                            for ci in range(c):
                                nc.tensor.matmul(
                                    o_ps[:, j, :], qpT(h, c),
                                    delta[:, ci, :],
                                    start=False, stop=(ci == c - 1),
                                    skip_group_check=True)
                    # normalize and write into o_out
                    rcp = small.tile([C, 4], F32, tag="rcp")
                    nc.vector.reciprocal(rcp, o_ps[:, :, D:D + 1])
                    nc.vector.tensor_tensor(o_out[:, 4 * g:4 * g + 4, :],
                                       o_ps[:, :, 0:D],
                                       rcp.to_broadcast((C, 4, D)),
                                       op=AL.mult)
                nc.scalar.dma_start(out=o_d[h], in_=o_out)

            for i in range(HPC + 2):
                if i < HPC:
                    emit_L(i)
                if 1 <= i <= HPC:
                    emit_F1(i - 1)
                if 2 <= i:
                    emit_B(i - 2)
                if 1 <= i <= HPC:
                    emit_F1b(i - 1)
                    emit_F2(i - 1)
                if i < HPC:
                    emit_SQ(i)
    nc.finalize()
    return nc


def make_in_maps(q, k, v, projection_matrix):
    qf = np.asarray(q, dtype=np.float32).reshape(B * H, S, D)
    kf = np.asarray(k, dtype=np.float32).reshape(B * H, S, D)
    vf = np.asarray(v, dtype=np.float32).reshape(B * H, S, D)
    pf = np.asarray(projection_matrix, dtype=np.float32)

    qkT = np.empty((B * H, 2 * D, S), dtype=ml_dtypes.bfloat16)
    qkT[:, 0:D, :] = qf.transpose(0, 2, 1).astype(ml_dtypes.bfloat16)
    qkT[:, D:2 * D, :] = kf.transpose(0, 2, 1).astype(ml_dtypes.bfloat16)
    # v: [h, s, d] -> [h, p, c, d+1] with ones column baked in
    vp = np.empty((B * H, C, T, D + 1), dtype=ml_dtypes.bfloat16)
    vp[:, :, :, 0:D] = vf.reshape(B * H, T, C, D).transpose(0, 2, 1, 3) \
        .astype(ml_dtypes.bfloat16)
    vp[:, :, :, D] = np.ones((), dtype=ml_dtypes.bfloat16)
    p2 = np.concatenate([pf, pf], axis=0).astype(ml_dtypes.bfloat16)

    in_maps = []
    for core in range(NCORES):
        sl = slice(core * HPC, (core + 1) * HPC)
        in_maps.append({"qkT": np.ascontiguousarray(qkT[sl]),
                        "v": np.ascontiguousarray(vp[sl]),
                        "proj": p2})
    return in_maps


_NC_CACHE = None


def kernel(q, k, v, projection_matrix):
    global _NC_CACHE
    if _NC_CACHE is None:
        _NC_CACHE = build_kernel()
    nc = _NC_CACHE

    in_maps = make_in_maps(q, k, v, projection_matrix)
    res = run_bass_kernel_spmd(nc, in_maps, list(range(NCORES)))
    out = np.concatenate([r["out"] for r in res.results], axis=0)
    # [h, p, c, d] -> [h, c*128+p, d]
    out = out.transpose(0, 2, 1, 3).reshape(B, H, S, D)
    return np.ascontiguousarray(out)


if __name__ == "__main__":
    rng = np.random.default_rng(0)
    inputs = {
        "q": rng.standard_normal((B, H, S, D)).astype(np.float32),
        "k": rng.standard_normal((B, H, S, D)).astype(np.float32),
        "v": rng.standard_normal((B, H, S, D)).astype(np.float32),
        "projection_matrix":
            (rng.standard_normal((D, M)) / np.sqrt(M)).astype(np.float32),
    }
    out = kernel(**inputs)
    print(out.shape, out.dtype)
